# revision 38
# baseline (speedup 1.0000x reference)
"""GuidedFilter (3-angle iterated boxfilter) on 8 trn2 NeuronCores.

Math: reference iterates  X <- X + (B_a(y) - B_a(X))/N_a  over 3 rotated-line
kernels B_a (17x5; the middle one is 17x1, 0 degrees).  With D = y - X this
is D <- (I - G_a B_a) D,  X_final = y - D_final.

The 0-degree step is COMPOSED into the first step:  pass 1 applies
(I - G1 B1)(I - G0 B0) as 5 horizontal taps with 33-tall banded vertical
profiles (exact per-row matrix products, including image-border rows);
pass 2 applies (I - G2 B2) with 17-tall bands.  Per (pass, chunk) the
update is ONE PSUM accumulation group:
  - slot C: plain bf16 matmul, lhsT = center-tap band incl identity, rhs = D
  - slots A,B: fp8 DoubleRow pair matmuls, each fusing TWO side taps
    (dx pairs (0,4) and (1,3)) against H = fp8(D), at bf16-slot cost.
Mapping: core (b, h) = (i//4, i%4) handles batch b, rows [512h, +512) via a
576-row slab split into 6 row-chunks of 128 at starts [0,96,192,288,384,448].
Evacuation: Act (pass 1 + even pass-2 chunks) and DVE (odd pass-2 chunks)
copy PSUM->SBUF bf16 (never the same PSUM tensor concurrently); DVE casts
bf16 tiles -> fp8 H tiles; DMA syncs 16-row chunk overlaps, DMA out.
The 4 left/right edge columns (where N varies per column) are recomputed
exactly on the host.
"""

import numpy as np
import ml_dtypes

M_IMG = 2048
N_IMG = 2048
BATCH = 2
H_SHARDS = 4
SH = 512            # rows per shard
SLAB = 576          # shard + 2*32
CW = 2052           # bf16 D tile width: 2 zero-pad cols each side
HCW = 2056          # fp8 H tile width: 4 zero-pad cols each side (aligned writes)
NCHUNK = 6
STARTS = [0, 96, 192, 288, 384, 448]   # chunk window starts within the slab
KH = 17
PC = 8
PAIRS = [(0, 4), (1, 3)]   # DoubleRow tap pairs (dx indices)
N_WC = 6                   # bf16 center lhsT count: pass*3+v
N_WP = 12                  # fp8 pair lhsT count: pass*6 + v*2 + pi
# pass p valid output rows within a 128-row window:
VLO = [16, 8]
VHI = [112, 120]
# pass-2 runs on 5 independent windows (slab-relative starts); each window's
# 128 input rows are restitched by DMA from the pass-1 valid rows ([16,112)
# of chunks j and j+1).
NP2 = 5
P2STARTS = [24, 128, 232, 336, 424]
V1STARTS = [0, 96, 448]      # weight-variant build rows, pass 1 (chunk geom)
V2STARTS = [24, 128, 424]    # weight-variant build rows, pass 2 (window geom)
# output row mapping per pass-2 window: (abs out row, p0, p1)
OUT_ROWS2 = [(0, 8, 120), (112, 16, 120), (216, 16, 120), (320, 16, 120),
             (424, 32, 120)]


def _restitch(j):
    """pass-2 tile j rows [0,128) = slab rows [w, w+128): n1 rows from
    chunk j partitions [src1,112) then n2 rows from chunk j+1 at [src2,...)."""
    w = P2STARTS[j]
    n1 = STARTS[j] + 112 - w
    src1 = w - STARTS[j]
    n2 = 128 - n1
    src2 = STARTS[j] + 112 - STARTS[j + 1]
    return n1, src1, n2, src2


def _band_matrix(kern, ginv, rows, a, dx, include_id):
    """[128,128] matrix M with M[m, m-8+t] -= g(m)*kern[a,t,dx], plus
    identity if include_id; rows outside the image are fully masked."""
    M = np.zeros((128, 128), np.float64)
    mask = (rows >= 0) & (rows < M_IMG)
    gv = np.where(mask, ginv[a][np.clip(rows, 0, M_IMG - 1)], 0.0)
    for m in range(128):
        if not mask[m]:
            continue
        for t in range(KH):
            k = m - PC + t
            if 0 <= k < 128 and kern[a, t, dx] != 0.0:
                M[m, k] -= gv[m] * kern[a, t, dx]
        if include_id:
            M[m, m] += 1.0
    return M


def _host_prep(X, y, kern_in, N_norm):
    kern = np.asarray(kern_in, np.float64)[:, 0]     # (3,17,5)
    N = np.asarray(N_norm, np.float64)[:, 0]         # (3,2048,2048)
    D0 = (np.asarray(y) - np.asarray(X))[:, 0]       # (2,2048,2048) f32

    ginv = 1.0 / N[:, :, N_IMG // 2]                 # (3,2048)

    in_maps = []
    for core in range(BATCH * H_SHARDS):
        b, h = core // H_SHARDS, core % H_SHARDS
        gs = SH * h - 32                             # global row of slab row 0

        d0s = np.zeros((SLAB, CW), np.float32)
        r0, r1 = max(0, gs), min(M_IMG, gs + SLAB)
        d0s[r0 - gs:r1 - gs, 2:2 + N_IMG] = D0[b, r0:r1]
        d0b = d0s.astype(ml_dtypes.bfloat16)

        # weight variants: v0 -> first chunk/window, v1 -> middle, v2 -> last
        wcs = np.zeros((N_WC, 128, 128), np.float64)
        wps = np.zeros((N_WP, 128, 2, 128), np.float64)
        for v in range(3):
            # pass 1: (I - G1 B1) @ (per-dx parts of I - G0 B0)
            rows1 = gs + V1STARTS[v] + np.arange(128)
            M1 = _band_matrix(kern, ginv, rows1, 1, 2, True)
            P = [M1 @ _band_matrix(kern, ginv, rows1, 0, dx, dx == 2)
                 for dx in range(5)]
            for p_ in P:
                p_[:VLO[0]] = 0.0
                p_[VHI[0]:] = 0.0
            # pass 2: angle 2 alone, on the pass-2 window geometry
            rows2 = gs + V2STARTS[v] + np.arange(128)
            A = [_band_matrix(kern, ginv, rows2, 2, dx, dx == 2)
                 for dx in range(5)]
            for a_ in A:
                a_[:VLO[1]] = 0.0
                a_[VHI[1]:] = 0.0
            for pa, mats in enumerate((P, A)):
                wcs[pa * 3 + v] = mats[2].T
                for pi, (dxL, dxR) in enumerate(PAIRS):
                    wps[pa * 6 + v * 2 + pi, :, 0, :] = mats[dxL].T
                    wps[pa * 6 + v * 2 + pi, :, 1, :] = mats[dxR].T

        wcs_p = np.ascontiguousarray(
            wcs.transpose(1, 0, 2).reshape(128, N_WC * 128)).astype(ml_dtypes.bfloat16)
        wps_p = np.ascontiguousarray(
            wps.reshape(N_WP, 128, 256).transpose(1, 0, 2).reshape(128, N_WP * 256)
        ).astype(ml_dtypes.float8_e4m3)
        in_maps.append({"d0b": d0b, "wcs": wcs_p, "wps": wps_p})
    return in_maps


def _build_program():
    import concourse.bass as bass
    from concourse import mybir

    f32 = mybir.dt.float32
    bf16 = mybir.dt.bfloat16
    fp8 = mybir.dt.float8e4
    DR = mybir.MatmulPerfMode.DoubleRow
    nc = bass.Bass("TRN2", target_bir_lowering=False)

    d0b_d = nc.dram_tensor("d0b", [SLAB, CW], bf16, kind="ExternalInput")
    wcs_d = nc.dram_tensor("wcs", [128, N_WC * 128], bf16, kind="ExternalInput")
    wps_d = nc.dram_tensor("wps", [128, N_WP * 256], fp8, kind="ExternalInput")
    xo = nc.dram_tensor("xo", [SH, N_IMG], bf16, kind="ExternalOutput")

    # Dt0: D0 pass-1 inputs; Dt1: D2 (pass-1 outputs, chunk geometry);
    # Dt2: pass-2 window tiles (D2 restitched, then D3 evac target);
    # Ht: fp8(D0); Ht2: fp8(D2 window).
    Dt0 = [nc.alloc_sbuf_tensor(f"d0_{c}", [128, CW], bf16) for c in range(NCHUNK)]
    Dt1 = [nc.alloc_sbuf_tensor(f"d1_{c}", [128, CW], bf16) for c in range(NCHUNK)]
    Dt2 = [nc.alloc_sbuf_tensor(f"d2_{j}", [128, CW], bf16) for j in range(NP2)]
    Ht = [nc.alloc_sbuf_tensor(f"h_{c}", [128, HCW], fp8) for c in range(NCHUNK)]
    Ht2 = [nc.alloc_sbuf_tensor(f"h2_{j}", [128, HCW], fp8) for j in range(NP2)]
    wcs = nc.alloc_sbuf_tensor("wcss", [128, N_WC * 128], bf16)
    wps = nc.alloc_sbuf_tensor("wpss", [128, N_WP * 256], fp8)
    warmr = nc.alloc_sbuf_tensor("warmr", [128, 528], bf16)
    ps = [nc.alloc_psum_tensor(f"ps{i}", [128, N_IMG], f32) for i in range(2)]

    def hpad_ap(t):
        return bass.AP(t, 0, [[HCW, 128], [HCW - 4, 2], [1, 4]])

    from contextlib import ExitStack
    with ExitStack() as stack:
        block = stack.enter_context(nc.Block(no_gpsimd_drain=True))
        sem = lambda n: stack.enter_context(nc.semaphore(n))
        sldw, spe, sact, spool, sout, swarm, sactp, sdvep = (
            sem("sldw"), sem("spe"), sem("sact"), sem("spool"),
            sem("sout"), sem("swarm"), sem("sactp"), sem("sdvep"))
        sldc = [sem(f"sldc{c}") for c in range(NCHUNK)]
        sld0l = sem("sld0l")
        sldp = sem("sldp")
        sldh = [sem(f"sldh{c}") for c in range(NCHUNK)]
        shf2 = [sem(f"shf2_{j}") for j in range(NP2)]

        @block.sync
        def _(sp):
            # priority order: center weights, chunk 0 (split in column
            # halves so the first matmuls can start on the left half), fp8
            # weights, then chunks 1-2.  Chunks 3-5 issue from the Scalar
            # queue once chunk 0 has landed.
            sp.dma_start(out=wcs[:, :], in_=wcs_d[:, :]).then_inc(sldw, 16)
            sp.dma_start(out=Dt0[0][:, 0:1026],
                         in_=d0b_d[STARTS[0]:STARTS[0] + 128, 0:1026]
                         ).then_inc(sld0l, 16)
            sp.dma_start(out=Dt0[0][:, 1026:CW],
                         in_=d0b_d[STARTS[0]:STARTS[0] + 128, 1026:CW]
                         ).then_inc(sldc[0], 16)
            sp.dma_start(out=wps[:, :], in_=wps_d[:, :]).then_inc(sldp, 16)
            for c in (1, 2):
                sp.dma_start(out=Dt0[c][:, :],
                             in_=d0b_d[STARTS[c]:STARTS[c] + 128, :]
                             ).then_inc(sldc[c], 16)
            # restitch pass-2 window tiles from pass-1 valid rows
            for j in range(NP2):
                n1, src1, n2, src2 = _restitch(j)
                sp.wait_ge(sact, j + 2)   # evacs of chunks j and j+1 done
                sp.dma_start(out=Dt2[j][0:n1, 2:2 + N_IMG],
                             in_=Dt1[j][src1:src1 + n1, 2:2 + N_IMG]
                             ).then_inc(shf2[j], 16)
                sp.dma_start(out=Dt2[j][n1:128, 2:2 + N_IMG],
                             in_=Dt1[j + 1][src2:src2 + n2, 2:2 + N_IMG]
                             ).then_inc(shf2[j], 16)
            # output DMAs: gated on both column-half evacs
            for j in range(NP2):
                o, p0, p1 = OUT_ROWS2[j]
                sp.wait_ge(sactp, j + 1)
                sp.wait_ge(sdvep, j + 1)
                sp.dma_start(out=xo[o:o + (p1 - p0), :],
                             in_=Dt2[j][p0:p1, 2:2 + N_IMG]).then_inc(sout, 16)
            sp.wait_ge(sout, 16 * NP2)

        @block.tensor
        def _(pe):
            pe.wait_ge(swarm, 1)
            for i in range(7):
                o = (i % 4) * 512
                pe.matmul(ps[0][:, o:o + 512], lhsT=warmr[:, 0:128],
                          rhs=warmr[:, 0:512], start=True, stop=True,
                          skip_group_check=True)
            for pa in range(2):
                NW = NCHUNK if pa == 0 else NP2
                for c in range(NW):
                    g = NCHUNK * pa + c
                    Din = Dt0[c] if pa == 0 else Dt2[c]
                    Hin = Ht[c] if pa == 0 else Ht2[c]
                    if pa == 0:
                        if c == 0:
                            pe.wait_ge(sldw, 16)
                            pe.wait_ge(sld0l, 16)
                        else:
                            pe.wait_ge(sldc[c], 16)
                    else:
                        pe.wait_ge(spool, c + 1)
                    if g >= 2:
                        gp, cp = divmod(g - 2, NCHUNK)
                        if gp == 0:
                            pe.wait_ge(sact, cp + 1)
                        else:
                            pe.wait_ge(sactp, cp + 1)
                            pe.wait_ge(sdvep, cp + 1)
                    v = {0: 0, NW - 1: 2}.get(c, 1)
                    wc_i = pa * 3 + v
                    for nt in range(4):
                        if pa == 0 and c == 0 and nt == 2:
                            pe.wait_ge(sldc[0], 16)
                        o = nt * 512
                        pe.matmul(ps[g % 2][:, o:o + 512],
                                  lhsT=wcs[:, wc_i * 128:(wc_i + 1) * 128],
                                  rhs=Din[:, o + 2:o + 514],
                                  start=True, stop=False,
                                  skip_group_check=True)
                    if pa == 0:
                        if c == 0:
                            pe.wait_ge(sldp, 16)
                        pe.wait_ge(sldh[c], 1)
                    for nt in range(4):
                        o = nt * 512
                        for pi, (dxL, dxR) in enumerate(PAIRS):
                            wp_i = pa * 6 + v * 2 + pi
                            mm = pe.matmul(
                                ps[g % 2][:, o:o + 512],
                                lhsT=bass.AP(wps, wp_i * 256,
                                             [[N_WP * 256, 128], [128, 2], [1, 128]]),
                                rhs=bass.AP(Hin, o + dxL + 2,
                                            [[HCW, 128], [dxR - dxL, 2], [1, 512]]),
                                start=False, stop=(pi == len(PAIRS) - 1),
                                perf_mode=DR, skip_group_check=True)
                        if nt == 3:
                            mm.then_inc(spe, 1)

        HALF = N_IMG // 2

        @block.scalar
        def _(act):
            # chunk 3-5 loads issue from the Scalar HW-DGE queue once the
            # critical chunk-0 bytes have cleared the engines
            act.wait_ge(sldc[0], 16)
            for c in range(3, NCHUNK):
                act.dma_start(out=Dt0[c][:, :],
                              in_=d0b_d[STARTS[c]:STARTS[c] + 128, :]
                              ).then_inc(sldc[c], 16)
            act.wait_ge(swarm, 1)
            act.copy(out=warmr[:, 516:528], in_=warmr[:, 0:12])  # act table
            for c in range(NCHUNK):
                act.wait_ge(spe, c + 1)
                act.copy(out=Dt1[c][:, 2:2 + N_IMG],
                         in_=ps[c % 2][:, :]).then_inc(sact, 1)
            # pass-2 evacuation, left column half (DVE does the right)
            for j in range(NP2):
                g = NCHUNK + j
                act.wait_ge(spe, g + 1)
                act.copy(out=Dt2[j][:, 2:2 + HALF],
                         in_=ps[g % 2][:, 0:HALF]).then_inc(sactp, 1)

        @block.vector
        def _(dve):
            dve.memset(warmr[:, 0:516], 0.0).then_inc(swarm, 1)
            for c in range(NCHUNK):
                dve.memset(hpad_ap(Ht[c]), 0.0)
            for j in range(NP2):
                dve.memset(hpad_ap(Ht2[j]), 0.0)
            for c in range(NCHUNK):
                dve.wait_ge(sldc[c], 16)
                if c == 0:
                    dve.wait_ge(sld0l, 16)
                dve.tensor_copy(out=Ht[c][:, 4:4 + N_IMG],
                                in_=Dt0[c][:, 2:2 + N_IMG]).then_inc(sldh[c], 1)
            def h2cast(j):
                dve.wait_ge(shf2[j], 32)
                dve.tensor_copy(out=Ht2[j][:, 4:4 + N_IMG],
                                in_=Dt2[j][:, 2:2 + N_IMG]).then_inc(spool, 1)

            def revac(j):
                g = NCHUNK + j
                dve.wait_ge(spe, g + 1)
                dve.tensor_copy(out=Dt2[j][:, 2 + HALF:2 + N_IMG],
                                in_=ps[g % 2][:, HALF:N_IMG]).then_inc(sdvep, 1)

            # interleave so revac(j) lands right after group NCHUNK+j while
            # each h2 cast still precedes the group that consumes it
            h2cast(0); h2cast(1); h2cast(2)
            revac(0)
            h2cast(3)
            revac(1)
            h2cast(4)
            revac(2); revac(3); revac(4)

    return nc


def _edge_strips(D0, kern, N):
    """Exact D3 on the 4 left / 4 right edge columns (f64 host compute)."""
    outs = []
    for side in range(2):
        W = 10
        if side == 0:
            s = D0[:, :, 0:W].astype(np.float64)
            colof = 0
        else:
            s = D0[:, :, N_IMG - W:].astype(np.float64)
            colof = N_IMG - W
        for a in range(3):
            sp = np.pad(s, ((0, 0), (8, 8), (2, 2)))
            B = np.zeros_like(s)
            for t in range(KH):
                for dx in range(5):
                    w = kern[a, t, dx]
                    if w != 0.0:
                        B += w * sp[:, t:t + M_IMG, dx:dx + W]
            Ncols = N[a, :, colof:colof + W]
            s = s - B / Ncols[None]
        outs.append(s[:, :, 0:4] if side == 0 else s[:, :, W - 4:])
    return outs[0], outs[1]


_LAST = None  # BassKernelResults of the most recent run (for test harness)


def kernel(X, y, kernel, N_norm):
    global _LAST
    from concourse.bass_utils import run_bass_kernel_spmd

    kern = np.asarray(kernel, np.float64)[:, 0]
    N = np.asarray(N_norm, np.float64)[:, 0]
    in_maps = _host_prep(X, y, kernel, N_norm)
    nc = _build_program()
    res = run_bass_kernel_spmd(nc, in_maps, list(range(BATCH * H_SHARDS)))
    _LAST = res

    yf = np.asarray(y)[:, 0].astype(np.float64)
    D3 = np.empty((BATCH, M_IMG, N_IMG), np.float64)
    for core in range(BATCH * H_SHARDS):
        b, h = core // H_SHARDS, core % H_SHARDS
        D3[b, SH * h:SH * h + SH, :] = res.results[core]["xo"].astype(np.float64)

    D0 = (np.asarray(y) - np.asarray(X))[:, 0]
    left, right = _edge_strips(D0, kern, N)
    D3[:, :, 0:4] = left
    D3[:, :, N_IMG - 4:] = right
    out = (yf - D3).astype(np.float32)
    return out[:, None]

